# revision 3
# baseline (speedup 1.0000x reference)
"""Trainium2 Bass kernel v2 for nn_Net_64982855188859 (ECC graph-conv net).

Same algebraic restructuring as v1 (w1c factored through x; a>=0 commuted
into eA through the zero-bias relu MLP; data-parallel over B=8 graphs), with
the device pipeline rebuilt around two measured HW facts:

 1. PSUM->SBUF eviction (only ACT+DVE can read PSUM, ~1 elem/lane/cycle)
    is the true bottleneck, so evicts are emitted as few, maximal-size
    ([128,1024]) contiguous ops, greedily load-balanced across ACT/DVE.
 2. PE tile_position row/col tiling runs matmuls concurrently on real HW,
    and per-instruction overhead dominates small matmuls. The edge MLP is
    restructured to 32 big matmuls total (vs 320):
      - h1: edges PAIRED (j = 2jj+d) via a [16,128] block-diagonal W so each
        streamed column carries 2 edges; 4-way row-tiled (K=16 per 32-row
        group), 16 matmuls of N=512 per layer.
      - h2: [128,64] block-diagonal W -> full K=128 matmuls, 2-way
        col-tiled, 16 matmuls of N=512 per layer, whose PSUM output IS the
        contraction chunk layout (plain contiguous evicts everywhere).

Layouts (per core = one graph; edge e=(j,i), pair jj=j//2, parity d=j%2,
column f = jj*128 + i < 8192):
  eA2_sb [128, 2048]: part 32*rt + 8*d + s, col 512*q + c  holds
         eAT[s, 2*jj+d, i] with f = 2048*q + 512*rt + c.
  h1 MM (q, rt): out[128 (d,u), 512] = W1a2[16,128].T @ eA2[32rt:+16, 512q:+512]
  h1s [128, 8192]: part 64*d + u, col f.
  h2 MM (T, cp): out[64cp:+64] = W1b2[128,64].T @ h1s[:, 1024T+512cp:+512]
  h2A [128, 4096]: part 64*cp + 32*d + k, col 512*T + 128*jjl + i
         = relu(h2) of edge j = 16*T + 8*cp + 2*jjl + d   (chunk t = 4T+jjl)
  xwr [128, 1024]: part (cp,d,k), col 32*t + c = xW[j(t,cp,d), k, c]
         (DRAM-roundtrip permutation, unchanged mechanism from v1)
  contraction: acc[32 c,128 i] += xwr[:,32t:+32].T @ h2A[:,128t:+128]
         one accumulation group with bias outer product + root term.
"""

import numpy as np

import concourse.bass as bass
import concourse.bacc as bacc
import concourse.mybir as mybir
import concourse.tile as tile
from concourse.bass_utils import run_bass_kernel_spmd

F32 = mybir.dt.float32
BF16 = mybir.dt.bfloat16
AF = mybir.ActivationFunctionType
ALU = mybir.AluOpType

B, N, F, S, C = 8, 128, 16, 8, 32

# wbf2 column layout ([128, 384] bf16): block-diagonal MLP weight stacks
W1A2 = slice(0, 128)      # [16,128] blockdiag w1a at rows 32rt+8d+s
W2A2 = slice(128, 256)
W1B2 = slice(256, 320)    # [128,64] blockdiag w1b at rows 64d+u
W2B2 = slice(320, 384)
WBF2_COLS = 384
# wpack column layout ([128, WCOLS] fp32): fp32 tail weights (as v1)
ROOT1 = slice(0, 32)      # rows 0:16
ROOT2 = slice(32, 64)     # rows 0:32
DW = slice(64, 128)       # rows 0:32
OW = slice(128, 138)      # rows 0:64
DB = slice(138, 139)      # rows 0:64  (column vector)
BIAS1 = slice(144, 176)   # row 0 (row vector)
BIAS2 = slice(176, 208)   # row 0
OB = slice(208, 218)      # row 0
WCOLS = 224


class _Balance:
    """Greedy ACT/DVE load balancer for PSUM->SBUF evict ops."""

    def __init__(self, nc):
        self.nc = nc
        self.t = [0.0, 0.0]  # est busy ns: [ACT, DVE]

    def _pick(self, cols):
        ca = (cols + 370) / 1.2
        cd = (cols + 240) / 0.96
        if self.t[0] + ca <= self.t[1] + cd:
            self.t[0] += ca
            return 0
        self.t[1] += cd
        return 1

    def relu(self, out, in_, cols):
        if self._pick(cols) == 0:
            self.nc.scalar.activation(out=out, in_=in_, func=AF.Relu)
        else:
            self.nc.vector.tensor_scalar_max(out, in_, 0.0)

    def copy(self, out, in_, cols):
        if self._pick(cols) == 0:
            self.nc.scalar.activation(out=out, in_=in_, func=AF.Copy)
        else:
            self.nc.vector.tensor_copy(out, in_)


def _xw_restructure(nc, bal, pool_ps, pool_xw, dram_pool, lhsT, rhs, name):
    """out[128 (cp,d,k), 1024 (t,c)] chunk-permuted xW via DRAM roundtrip.
    All DMAs ride the gpsimd (SWDGE) queue to keep SP/ACT queues free."""
    ps = pool_ps.tile([128, 1024], F32, tag="ps", name=f"{name}_ps")
    for h in range(2):
        nc.tensor.matmul(out=ps[:, 512 * h:512 * h + 512], lhsT=lhsT,
                         rhs=rhs[:, 512 * h:512 * h + 512], start=True, stop=True)
    flat = pool_xw.tile([128, 1024], BF16, tag=f"{name}_flat")
    bal.copy(flat[:, 0:512], ps[:, 0:512], 512)
    bal.copy(flat[:, 512:1024], ps[:, 512:1024], 512)
    dscratch = dram_pool.tile([128, 1024], BF16, tag=f"{name}_dram")
    nc.gpsimd.dma_start(out=dscratch[:, :], in_=flat[:, :])
    restr = pool_xw.tile([128, 1024], BF16, tag=f"{name}_restr")
    dap = dscratch[:, :]
    # xwr[32*b2 + k, 32*u + c] = xW[32*b2 + u, 32*k + c]
    for b2 in range(4):
        src = bass.AP(
            tensor=dap.tensor, offset=dap.offset + 32 * b2 * 1024,
            ap=[[32, 32], [1024, 32], [1, 32]],
        )
        nc.gpsimd.dma_start(
            out=restr[32 * b2:32 * b2 + 32, :].rearrange("k (u c) -> k u c",
                                                         c=32),
            in_=src,
        )
    return restr


def build_nc(loop_n: int | None = None):
    nc = bacc.Bacc("TRN2", target_bir_lowering=False, debug=False)
    eA2_d = nc.dram_tensor("eA2", [64, 2048], BF16, kind="ExternalInput").ap()
    xc_d = nc.dram_tensor("xc", [17, 128], F32, kind="ExternalInput").ap()
    xcb_d = nc.dram_tensor("xcb", [16, 128], BF16, kind="ExternalInput").ap()
    wp_d = nc.dram_tensor("wpack", [128, WCOLS], F32, kind="ExternalInput").ap()
    wbf_d = nc.dram_tensor("wbf2", [128, WBF2_COLS], BF16,
                           kind="ExternalInput").ap()
    w1p_d = nc.dram_tensor("w1p", [16, 1024], BF16, kind="ExternalInput").ap()
    w2p_d = nc.dram_tensor("w2p", [32, 1024], BF16, kind="ExternalInput").ap()
    out_d = nc.dram_tensor("out", [1, 10], F32, kind="ExternalOutput").ap()

    with tile.TileContext(nc) as tc:
        with (
            tc.tile_pool(name="consts", bufs=1) as consts,
            tc.tile_pool(name="eA2", bufs=1) as pool_eA,
            tc.tile_pool(name="ps", bufs=3, space="PSUM") as pool_ps,
            tc.tile_pool(name="acc", bufs=2, space="PSUM") as pool_acc,
            tc.tile_pool(name="h1s", bufs=2) as pool_h1s,
            tc.tile_pool(name="h2A", bufs=2) as pool_h2A,
            tc.tile_pool(name="xw", bufs=1) as pool_xw,
            tc.tile_pool(name="misc", bufs=1) as pool_misc,
            tc.tile_pool(name="dram", bufs=1, space="DRAM") as dram_pool,
        ):
            def mlp_layer(bal, eA2_sb, wb_sb, wa_cols, wb_cols, acc, xwr_sb,
                          h1s, h2A, lname):
                """Edge MLP + interleaved contraction for one ECC layer."""
                for q in range(4):
                    ht = [pool_ps.tile([128, 1024], F32, tag="ps",
                                       name=f"{lname}h1{q}a"),
                          pool_ps.tile([128, 1024], F32, tag="ps",
                                       name=f"{lname}h1{q}b")]
                    for rt in range(4):
                        nc.tensor.matmul(
                            out=ht[rt // 2][:, 512 * (rt % 2):512 * (rt % 2) + 512],
                            lhsT=wb_sb[32 * rt:32 * rt + 16, wa_cols],
                            rhs=eA2_sb[32 * rt:32 * rt + 16, 512 * q:512 * q + 512],
                            start=True, stop=True,
                            tile_position=(32 * rt, 0),
                        )
                    for h in range(2):
                        bal.relu(h1s[:, 2048 * q + 1024 * h:
                                     2048 * q + 1024 * h + 1024],
                                 ht[h][:, :], 1024)
                    gt = pool_ps.tile([128, 1024], F32, tag="ps",
                                      name=f"{lname}h2{q}")
                    for tl in range(2):
                        for cp in range(2):
                            nc.tensor.matmul(
                                out=gt[64 * cp:64 * cp + 64,
                                       512 * tl:512 * tl + 512],
                                lhsT=wb_sb[:, wb_cols],
                                rhs=h1s[:, 2048 * q + 1024 * tl + 512 * cp:
                                        2048 * q + 1024 * tl + 512 * cp + 512],
                                start=True, stop=True,
                                tile_position=(0, 64 * cp),
                            )
                    bal.relu(h2A[:, 1024 * q:1024 * q + 1024], gt[:, :], 1024)
                    # contraction chunks of this round (t = 8q .. 8q+7)
                    for t in range(8 * q, 8 * q + 8):
                        nc.tensor.matmul(
                            out=acc[:, :],
                            lhsT=xwr_sb[:, 32 * t:32 * t + 32],
                            rhs=h2A[:, 128 * t:128 * t + 128],
                            start=False, stop=False, skip_group_check=True,
                        )

            # ---- weights: loaded ONCE, outside the timing loop ----
            wb_sb = consts.tile([128, WBF2_COLS], BF16)
            nc.sync.dma_start(out=wb_sb[:, :], in_=wbf_d)
            w1p_sb = consts.tile([16, 1024], BF16)
            nc.sync.dma_start(out=w1p_sb[:, :], in_=w1p_d)
            w2p_sb = consts.tile([32, 1024], BF16)
            nc.scalar.dma_start(out=w2p_sb[:, :], in_=w2p_d)
            wp_sb = consts.tile([128, WCOLS], F32)
            nc.scalar.dma_start(out=wp_sb[:, :], in_=wp_d)
            ones_sb = consts.tile([1, 128], F32)
            nc.vector.memset(ones_sb[:, :], 1.0)
            # Warm the ACT table set (Exp's set also holds Relu/Copy) so the
            # in-loop activations don't reload it.
            warm = consts.tile([1, 1], F32)
            nc.scalar.activation(out=warm[:, :], in_=ones_sb[0:1, 0:1],
                                 func=AF.Exp)

            def body():
                bal = _Balance(nc)
                # ---- per-iteration data loads, split across DMA queues ----
                eA2_sb = pool_eA.tile([128, 2048], BF16, tag="eA2")
                for rt in range(4):
                    eng = nc.sync if rt % 2 == 0 else nc.scalar
                    eng.dma_start(out=eA2_sb[32 * rt:32 * rt + 16, :],
                                  in_=eA2_d[16 * rt:16 * rt + 16, :])
                xcb_sb = pool_eA.tile([16, 128], BF16, tag="xcb")
                nc.sync.dma_start(out=xcb_sb[:, :], in_=xcb_d)
                xc_sb = pool_eA.tile([17, 128], F32, tag="xc")
                nc.scalar.dma_start(out=xc_sb[:, :], in_=xc_d)
                xT = xc_sb[0:16, :]
                mask_rep = pool_eA.tile([32, 128], F32, tag="mask")
                nc.gpsimd.dma_start(
                    out=mask_rep[:, :],
                    in_=bass.AP(tensor=xc_d.tensor, offset=16 * 128,
                                ap=[[0, 32], [1, 128]]),
                )

                # ---- xW restructured (only needs x; contraction-only dep) ----
                xwr_sb = _xw_restructure(nc, bal, pool_ps, pool_xw, dram_pool,
                                         xcb_sb[:, :], w1p_sb[:, :], "xw")

                # ---- layer 1 ----
                h1s1 = pool_h1s.tile([128, 8192], BF16, tag="h1s")
                h2A1 = pool_h2A.tile([128, 4096], BF16, tag="h2A")
                acc1 = pool_acc.tile([32, 128], F32, tag="acc")
                nc.tensor.matmul(out=acc1[:, :], lhsT=wp_sb[0:1, BIAS1],
                                 rhs=ones_sb[:, :], start=True, stop=False,
                                 skip_group_check=True)
                mlp_layer(bal, eA2_sb, wb_sb, W1A2, W1B2, acc1, xwr_sb,
                          h1s1, h2A1, "l1")
                nc.tensor.matmul(out=acc1[:, :], lhsT=wp_sb[0:16, ROOT1],
                                 rhs=xT, start=False, stop=True,
                                 skip_group_check=True)
                z_sb = pool_misc.tile([32, 128], F32, tag="z")
                nc.vector.tensor_mul(z_sb[:, :], acc1[:, :], mask_rep[:, :])
                y1_sb = pool_misc.tile([32, 128], F32, tag="y1")
                nc.vector.scalar_tensor_tensor(
                    out=y1_sb[:, :], in0=z_sb[:, :], scalar=0.05,
                    in1=z_sb[:, :], op0=ALU.mult, op1=ALU.max)
                y1_bf = pool_misc.tile([32, 128], BF16, tag="y1bf")
                nc.vector.tensor_copy(y1_bf[:, :], y1_sb[:, :])

                # ---- yW restructured ----
                ywr_sb = _xw_restructure(nc, bal, pool_ps, pool_xw, dram_pool,
                                         y1_bf[:, :], w2p_sb[:, :], "yw")

                # ---- layer 2 ----
                h1s2 = pool_h1s.tile([128, 8192], BF16, tag="h1s")
                h2A2 = pool_h2A.tile([128, 4096], BF16, tag="h2A")
                acc2 = pool_acc.tile([32, 128], F32, tag="acc")
                nc.tensor.matmul(out=acc2[:, :], lhsT=wp_sb[0:1, BIAS2],
                                 rhs=ones_sb[:, :], start=True, stop=False,
                                 skip_group_check=True)
                mlp_layer(bal, eA2_sb, wb_sb, W2A2, W2B2, acc2, ywr_sb,
                          h1s2, h2A2, "l2")
                nc.tensor.matmul(out=acc2[:, :], lhsT=wp_sb[0:32, ROOT2],
                                 rhs=y1_sb[:, :], start=False, stop=True,
                                 skip_group_check=True)

                # ---- finish layer 2 + head ----
                r2_sb = pool_misc.tile([32, 128], F32, tag="r2")
                nc.scalar.activation(out=r2_sb[:, :], in_=acc2[:, :],
                                     func=AF.Relu)
                h2f_sb = pool_misc.tile([32, 128], F32, tag="h2f")
                gv_sb = pool_misc.tile([32, 1], F32, tag="gv")
                nc.vector.tensor_mul(h2f_sb[:, :], r2_sb[:, :], mask_rep[:, :])
                nc.vector.reduce_sum(out=gv_sb[:, :], in_=h2f_sb[:, :],
                                     axis=mybir.AxisListType.X)
                d_ps = pool_acc.tile([64, 1], F32, tag="acc")
                nc.tensor.matmul(out=d_ps[:, :], lhsT=wp_sb[0:32, DW],
                                 rhs=gv_sb[:, :], start=True, stop=True)
                d_sb = pool_misc.tile([64, 1], F32, tag="d")
                nc.scalar.activation(out=d_sb[:, :], in_=d_ps[:, :],
                                     func=AF.Relu, bias=wp_sb[0:64, DB],
                                     scale=1.0 / 128.0)
                lg_ps = pool_acc.tile([1, 10], F32, tag="acc")
                nc.tensor.matmul(out=lg_ps[:, :], lhsT=d_sb[:, :],
                                 rhs=wp_sb[0:64, OW], start=True, stop=True)
                lg_sb = pool_misc.tile([1, 10], F32, tag="lg")
                nc.vector.tensor_add(lg_sb[:, :], lg_ps[:, :], wp_sb[0:1, OB])
                ex_sb = pool_misc.tile([1, 10], F32, tag="ex")
                ssum = pool_misc.tile([1, 1], F32, tag="ssum")
                nc.scalar.activation(out=ex_sb[:, :], in_=lg_sb[:, :],
                                     func=AF.Exp, accum_out=ssum[:, :])
                rs_sb = pool_misc.tile([1, 1], F32, tag="rs")
                nc.vector.reciprocal(rs_sb[:, :], ssum[:, :])
                probs = pool_misc.tile([1, 10], F32, tag="probs")
                nc.vector.tensor_scalar(
                    out=probs[:, :], in0=ex_sb[:, :], scalar1=rs_sb[0:1, 0:1],
                    scalar2=None, op0=ALU.mult)
                nc.sync.dma_start(out=out_d, in_=probs[:, :])

            if loop_n is not None and loop_n > 1:
                with tc.For_i(0, loop_n, 1, hint_engines=(
                        mybir.EngineType.PE, mybir.EngineType.DVE,
                        mybir.EngineType.Activation, mybir.EngineType.SP)):
                    body()
            else:
                body()
    nc.compile()
    return nc


def prep_inputs(x, a, e, w1a, b1a, w1b, b1b, w1c, b1c, root1, bias1,
                w2a, b2a, w2b, b2b, w2c, b2c, root2, bias2, dw, db, ow, ob):
    """Host-side shard + layout prep. Returns in_maps (one per core)."""
    x = np.asarray(x, np.float32)
    a = np.asarray(a, np.float32)
    e = np.asarray(e, np.float32)
    # These biases are structurally zero in this problem (jnp.zeros in
    # setup_inputs); the device program relies on it (see module docstring).
    for b_ in (b1a, b1b, b1c, b2a, b2b, b2c):
        assert np.abs(np.asarray(b_)).max() == 0.0, "nonzero MLP bias unsupported"

    import ml_dtypes
    bf16 = ml_dtypes.bfloat16

    wbf2 = np.zeros((128, WBF2_COLS), bf16)
    w1a_ = np.asarray(w1a).astype(bf16)
    w2a_ = np.asarray(w2a).astype(bf16)
    for rt in range(4):
        for d in range(2):
            r0 = 32 * rt + 8 * d
            wbf2[r0:r0 + 8, 64 * d:64 * d + 64] = w1a_
            wbf2[r0:r0 + 8, 128 + 64 * d:128 + 64 * d + 64] = w2a_
    w1b_ = np.asarray(w1b).astype(bf16)
    w2b_ = np.asarray(w2b).astype(bf16)
    for d in range(2):
        wbf2[64 * d:64 * d + 64, 256 + 32 * d:256 + 32 * d + 32] = w1b_
        wbf2[64 * d:64 * d + 64, 320 + 32 * d:320 + 32 * d + 32] = w2b_

    wpack = np.zeros((128, WCOLS), np.float32)
    wpack[0:16, ROOT1] = np.asarray(root1)
    wpack[0:32, ROOT2] = np.asarray(root2)
    wpack[0:32, DW] = np.asarray(dw)
    wpack[0:64, OW] = np.asarray(ow)
    wpack[0:64, DB] = np.asarray(db).reshape(64, 1)
    wpack[0:1, BIAS1] = np.asarray(bias1).reshape(1, 32)
    wpack[0:1, BIAS2] = np.asarray(bias2).reshape(1, 32)
    wpack[0:1, OB] = np.asarray(ob).reshape(1, 10)

    w1p = np.ascontiguousarray(
        np.asarray(w1c).reshape(32, 16, 32).transpose(1, 0, 2).reshape(16, 1024)
    ).astype(bf16)
    w2p = np.ascontiguousarray(
        np.asarray(w2c).reshape(32, 32, 32).transpose(1, 0, 2).reshape(32, 1024)
    ).astype(bf16)

    # slot jj' -> edge j: j = 64*cp + 32*d + 4*T + jjl
    # (T = jj'//8, cp = (jj'%8)//4, jjl = jj'%4)
    jjp = np.arange(64)
    Jmap = np.empty((2, 64), np.int64)
    for d in range(2):
        Jmap[d] = 64 * ((jjp % 8) // 4) + 32 * d + 4 * (jjp // 8) + (jjp % 4)

    in_maps = []
    for g in range(B):
        eAT = (e[g] * a[g][..., None]).transpose(2, 1, 0)  # [S, j, i]
        X = np.stack([eAT[:, Jmap[0], :], eAT[:, Jmap[1], :]],
                     axis=1)  # [s, d, jj', i]
        X2 = X.reshape(S, 2, 4, 4, 512)  # [s, d, q, rt, c]
        eA2 = np.ascontiguousarray(
            X2.transpose(3, 1, 0, 2, 4).reshape(64, 2048)).astype(bf16)
        xc = np.ascontiguousarray(x[g].T)  # [17, 128]; rows 0:16 feats, 16 mask
        xcb = np.ascontiguousarray(x[g].T[0:16]).astype(bf16)
        in_maps.append(dict(eA2=eA2, xc=xc, xcb=xcb, wpack=wpack, wbf2=wbf2,
                            w1p=w1p, w2p=w2p))
    return in_maps


_NC_CACHE = {}


def _get_nc(loop_n=None):
    key = loop_n
    if key not in _NC_CACHE:
        _NC_CACHE[key] = build_nc(loop_n)
    return _NC_CACHE[key]


def kernel(**inputs) -> np.ndarray:
    in_maps = prep_inputs(**inputs)
    nc = _get_nc()
    # The axon-tunneled device occasionally reports a transient
    # "exec unit unrecoverable" on the first dispatch after idle; a retry on
    # a fresh dispatch has always succeeded, so try up to 3 times.
    last = None
    for _ in range(3):
        try:
            res = run_bass_kernel_spmd(nc, in_maps, core_ids=list(range(B)))
            out = np.concatenate(
                [res.results[g]["out"] for g in range(B)], axis=0)
            return out.astype(np.float32)
        except Exception as ex:  # noqa: BLE001
            last = ex
    raise last
